# revision 58
# baseline (speedup 1.0000x reference)
"""Masked dot-product attention (B=16, Q=K=2048, D=64) on 8 Trainium2 cores.

out = softmax(Q K^T / sqrt(64) + mask(valid_lens)) V, reproducing
reference.py's masked_softmax exactly (to fp16-matmul precision).

Sharding / load balance
-----------------------
Work units are (batch, 512-wide q-block): 64 units whose cost is
nk(b) = ceil(valid_len[b]/128) k-tiles. Units are sorted by nk descending and
dealt round-robin into 8 slots x 8 cores, so every core runs the *same*
static SPMD program (slot j processes NK_j = max-nk-of-its-rank-group
k-tiles) while the host packs each core's own data. Per-core inputs arrive
as packed [128, *] fp16 buffers per slot: [Q^T dup | K^T half-packed] and
[V_aug] (see _xw). Q|K and V ride separate DMAs, with V transfers trailing
so the early (ramp-limited) DMA bandwidth all feeds the S-matmul stream;
the Sync queue carries ONLY input transfers, all epilogue DMAs ride
GpSimd (epilogue producer-waits on Sync were observed head-of-line
blocking V issues and stalling O-matmuls by several us).

Device pipeline (inputs fp16; PSUM accumulates fp32)
----------------------------------------------------
The ACT engine is the roofline: every score flows through one EXP
ACTIVATE at 1 elem/cycle/lane @ 1.2 GHz (~35us/core for 67 k-tiles).
Everything else is structured to keep ACT saturated:
  PE : S^T[128k, 512q] per k-tile = matmul(lhsT=K^T-tile, rhs=Q^T),
       contraction d=64, alternating k-tiles on PE row groups 0-63/64-127
  ACT: P = exp(S^T/8), one ACTIVATE over the 3-bank PSUM group
  PE : O^T_aug[65, 512q] += matmul(lhsT=V_aug-tile[128,65], rhs=P-slice)
O-matmuls lag one group behind S so the PE queue never head-of-line
blocks on exp. V_aug = [V | 1] with rows >= valid_len zeroed by the host
(exact masking, free denominator in row 64). A burst of dependency-free
warm-up matmuls at t=0 plus a cold-phase O-deferral (the first
COLD_GROUPS groups' O-matmuls are stashed and drained two-per-group once
the PE HAM clock-gate has had a full activity window to open to 2.4 GHz;
cold S-matmuls alone keep pace with exp) protect the exp stream from the
1.2 GHz cold clock. The slot order (three smallest first, then largest
descending, biggest last) gets the first EXP issued as early as the
first small DMA allows while the big transfers stream in behind.

Division epilogue
-----------------
Positions 0-5 (processing order): denominator rows are batched on DVE
([3,512] RECIPROCAL x2 - iterative divide is FD-serial, so batching
across partitions amortizes it), broadcast across the 64 d-partitions via
a DRAM-bounce DMA, then multiplied + DMA'd out, all hidden under later
units' exp stream. Positions 6 and 7 instead reciprocate on the ACT
engine as r = exp(-ln(d)) (both functions live in the
natural_log_exp_and_others table set - no table switch; _Bacc pins the
selection), broadcast by a ones-column PE matmul in fp16 (5e-4 rel err,
far under tolerance); their PSUM accumulators stay live to the tail, so
no DVE divide or bounce sits on the critical path after the last exp.
The host transposes O^T -> O while unsharding.
"""

import sys

if "/opt/trn_rl_repo" not in sys.path:
    sys.path.insert(0, "/opt/trn_rl_repo")

import numpy as np

import bass_rust as _bass_rust
import concourse.bass as bass
import concourse.mybir as mybir
import concourse.tile as tile
from concourse import bacc
from concourse.bass_utils import run_bass_kernel_spmd
from concourse.hw_specs import get_activation_tables

B, Q, KLEN, D = 16, 2048, 2048, 64
QB = 512                      # q-block width per work unit
NCORES = 8
NSLOTS = (B * (Q // QB)) // NCORES   # 8 slots per core
KT = 128                      # k-tile height
GK = 3                        # k-tiles per exp/ACT group (3 PSUM banks)
NWARM = 8                     # dependency-free PE warm-up matmuls
COLD_GROUPS = 4               # exp groups whose O-matmuls defer to warm PE
F32 = mybir.dt.float32
F16 = mybir.dt.float16
NPF16 = np.float16
AF = mybir.ActivationFunctionType

LAST_RESULTS = None           # BassKernelResults of the most recent run

_cache: dict = {}


class _Bacc(bacc.Bacc):
    """Bacc whose activation-table fixpoint keeps every ACTIVATE on one
    set: Exp and Ln both live in natural_log_exp_and_others, but the
    default selector binds Exp to exp_and_others and then pays two ~1.3us
    ACT_TABLE_LOAD switches around the tail's Ln. Restricting Exp/Ln to
    the combined set (table order/indices preserved - the set id is the
    index into act_info.json) yields a single load."""

    def insert_act_table_loads(self):
        has_activation = any(
            isinstance(i, mybir.InstActivation)
            for b in self.main_func.blocks
            for i in b.instructions
        )
        if not has_activation:
            return
        tables = []
        for name, fns in get_activation_tables(self.m.arch).items():
            if name != "natural_log_exp_and_others":
                fns = fns - {AF.Exp, AF.Ln}
            tables.append((name, fns))
        _bass_rust.insert_act_table_loads(self, tables)


def _schedule(valid_lens):
    """Static work schedule from valid_lens (host-known at call time)."""
    nk = [max(1, -(-int(v) // KT)) for v in valid_lens]
    units = [(b, qb) for b in range(B) for qb in range(Q // QB)]
    units.sort(key=lambda u: (-nk[u[0]], u))
    slots_nk = [nk[units[NCORES * j][0]] for j in range(NSLOTS)]
    assign = [[units[NCORES * j + c] for j in range(NSLOTS)] for c in range(NCORES)]
    offs = np.concatenate([[0], np.cumsum(slots_nk)]).tolist()
    return nk, slots_nk, offs, assign


def _order(slots_nk):
    """Processing order: two smallest first (their little DMAs land fast,
    so the exp stream starts early), then the rest descending so the
    serial input-DMA stream always runs ahead of compute, and the biggest
    slot last: the two DVE reciprocal batches then have the whole final
    slot's stream to hide under, leaving only the ACT-based division of
    the final unit on the tail."""
    asc = sorted(range(NSLOTS), key=lambda j: (slots_nk[j], j))
    return [asc[0], asc[1], asc[2], asc[6], asc[4], asc[3], asc[5], asc[7]]


def _xw(w):
    """Per-slot packed widths: [Q^T dup | K^T half-packed] + [V_aug].

    K^T tiles alternate between partition halves (even k-tile i in rows
    0-63 at column block i//2, odd in rows 64-127) - half the K DMA bytes
    of a full duplication while keeping the PE row-group pairing; only
    the small Q^T block is duplicated."""
    kk = (w + 1) // 2
    return QB + kk * KT, w * 65


def _build(slots_nk, offs):
    """Build + compile the single SPMD program for the given slot profile."""
    order = _order(slots_nk)
    xw = [sum(_xw(w)) for w in slots_nk]
    xoffs = np.concatenate([[0], np.cumsum([xw[j] for j in order])]).tolist()
    # reciprocal batches over unit positions (processing order); the final
    # two positions divide on ACT instead (see docstring)
    rbatches = [[0, 1, 2], [3, 4, 5]]
    ACT_DIV = (6, 7)

    nc = _Bacc()
    data_d = nc.dram_tensor("data", [2 * D, xoffs[-1]], F16,
                            kind="ExternalInput").ap()
    out_d = nc.dram_tensor("out", [NSLOTS, D, QB], F32, kind="ExternalOutput").ap()

    with tile.TileContext(nc) as tc:
        with (
            tc.tile_pool(name="spool", bufs=8) as spool,
            tc.tile_pool(name="vpool", bufs=8) as vpool,
            tc.tile_pool(name="ppool", bufs=8) as ppool,
            tc.tile_pool(name="epool", bufs=3) as epool,
            tc.tile_pool(name="gpool", bufs=1) as gpool,
            tc.tile_pool(name="opool", bufs=8) as opool,
            tc.tile_pool(name="dpool", bufs=2, space="DRAM") as dpool,
            tc.tile_pool(name="psum_s", bufs=2, space="PSUM") as psum_s,
            tc.tile_pool(name="psum_o", bufs=2, space="PSUM") as psum_o,
        ):
            dn_tiles = {}
            for bi, ub in enumerate(rbatches):
                dn_tiles[bi] = gpool.tile([len(ub), QB], F32, name=f"dn{bi}",
                                          tag=f"dn{bi}")
            ones_sb = gpool.tile([1, D], F16, name="ones", tag="ones")
            nc.vector.memset(ones_sb, 1.0)

            # PE warm-up: dependency-free matmuls from t=0 keep the PE busy
            # through the HAM activity window so the clock-gate opens to
            # 2.4 GHz before (or soon after) the first real matmul.
            warm_row = gpool.tile([1, QB], F16, name="warm_row", tag="warm_row")
            nc.vector.memset(warm_row, 1.0)
            warm_ps = psum_o.tile([65, QB], F32, name="warm_ps", tag="po")
            for _ in range(NWARM):
                nc.tensor.matmul(warm_ps[0:64, :], lhsT=ones_sb, rhs=warm_row,
                                 start=True, stop=True)

            o_tiles = {}
            div_stage2 = []   # (ub, tile) waiting for multiplies

            def div_stage1(bi, ub, dn):
                # reciprocal + partition-broadcast via DRAM bounce (DRAM is
                # flat, so the read-back can replicate across partitions
                # with a stride-0 leading dim), hidden under later units'
                # exp stream. Out-hop on Sync, in-hop on the otherwise-idle
                # GpSimd queue so neither blocks input-DMA issues.
                r_sb = epool.tile([len(ub), QB], F32, tag="r")
                nc.vector.reciprocal(r_sb, dn)
                scratch = dpool.tile([len(ub), QB], F32, tag="scr")
                nc.gpsimd.dma_start(out=scratch, in_=r_sb)
                rb_sb = epool.tile([D, len(ub), QB], F32, tag="rb")
                bcast_src = bass.AP(
                    tensor=scratch.tensor,
                    offset=scratch.offset,
                    ap=[[0, D]] + [list(a) for a in scratch.ap],
                )
                nc.gpsimd.dma_start(out=rb_sb, in_=bcast_src)
                div_stage2.append((list(ub), rb_sb))

            def emit_stage2(ub, rtile):
                for ui, jj in enumerate(ub):
                    oo_sb = opool.tile([D, QB], F32, tag="oo")
                    nc.vector.tensor_mul(oo_sb, o_tiles[jj], rtile[:, ui, :])
                    nc.gpsimd.dma_start(out=out_d[order[jj]], in_=oo_sb)

            def flush_stage2():
                while div_stage2:
                    emit_stage2(*div_stage2.pop(0))

            slot_ctx = {}

            def open_qk(jidx):
                j = order[jidx]
                w = slots_nk[j]
                wqk, wv = _xw(w)
                qk_sb = spool.tile([2 * D, wqk], F16, tag="xqk")
                nc.sync.dma_start(
                    out=qk_sb,
                    in_=data_d[:, xoffs[jidx]:xoffs[jidx] + wqk])
                po = psum_o.tile([65, QB], F32, tag="po")
                slot_ctx[jidx] = [qk_sb, None, po, w]

            def open_v(jidx):
                j = order[jidx]
                w = slots_nk[j]
                wqk, wv = _xw(w)
                xv_sb = vpool.tile([2 * D, wv], F16, tag="xv")
                nc.sync.dma_start(
                    out=xv_sb,
                    in_=data_d[:, xoffs[jidx] + wqk:xoffs[jidx] + wqk + wv])
                slot_ctx[jidx][1] = xv_sb

            act_div_units = []   # (jidx, po) divided on ACT at the tail

            def close_slot(jidx, last=False):
                # flush every pending multiply batch first (their bounce
                # DMAs were issued at least one slot ago), then start this
                # unit's denominator chain as early as possible
                _, _, po, _ = slot_ctx[jidx]
                flush_stage2()
                if jidx in ACT_DIV:
                    # the last two positions divide on the ACT engine after
                    # the exp stream ends: r = exp(-ln(d)) (same activation
                    # table set as exp - no table switch), broadcast across
                    # the 64 d-partitions by a ones-column PE matmul in
                    # fp16. Their PSUM accumulators stay live (nothing
                    # reuses those banks), so only the numerator copy runs
                    # here; no DVE reciprocal or DRAM bounce sits on the
                    # critical tail.
                    oa_sb = gpool.tile([D, QB], F32, name=f"oa{jidx}",
                                       tag=f"oa{jidx}")
                    nc.vector.tensor_copy(oa_sb, po[0:64, :])
                    o_tiles[jidx] = oa_sb
                    act_div_units.append((jidx, po))
                    if last:
                        for jj, pp in act_div_units:
                            lnd = epool.tile([1, QB], F32, tag=f"lnd{jj}")
                            nc.scalar.activation(lnd, pp[64:65, :], AF.Ln)
                            r16 = epool.tile([1, QB], F16, tag=f"r16{jj}")
                            nc.scalar.activation(r16, lnd, AF.Exp,
                                                 scale=-1.0)
                            # broadcast target comes from the psum_o pool:
                            # the rotation lands it exactly on this unit's
                            # own accumulator bank, whose readers (the ln
                            # and the numerator copy) are this chain's own
                            # upstream - taking a psum_s buf here instead
                            # was observed stalling the last unit's
                            # S-matmuls ~2us mid-stream (the scheduler
                            # parks the broadcast consumer at the tail
                            # while the ps pool waits on it)
                            bc = psum_o.tile([65, QB], F32, tag="po")
                            nc.tensor.matmul(bc[0:D, :], lhsT=ones_sb,
                                             rhs=r16, start=True, stop=True)
                            oo_sb = opool.tile([D, QB], F32, tag="oo")
                            nc.vector.tensor_mul(oo_sb, o_tiles[jj],
                                                 bc[0:D, :])
                            nc.gpsimd.dma_start(out=out_d[order[jj]],
                                                in_=oo_sb)
                    return
                bi = next(i for i, ub in enumerate(rbatches) if jidx in ub)
                ri = rbatches[bi].index(jidx)
                # one [65,512] PSUM->SBUF copy moves numerator + denominator
                # together (frees the PSUM bank in one op); the denominator
                # row then hops SBUF->SBUF into the batch tile
                oa_sb = gpool.tile([65, QB], F32, name=f"oa{jidx}",
                                   tag=f"oa{jidx}")
                nc.vector.tensor_copy(oa_sb, po)
                o_tiles[jidx] = oa_sb[0:D, :]
                # the Sync queue carries ONLY input transfers: any epilogue
                # DMA there lets the static scheduler park input issues
                # behind a producer wait (observed stalling V transfers,
                # and with them O-matmuls, by several us)
                nc.gpsimd.dma_start(out=dn_tiles[bi][ri:ri + 1, :],
                                    in_=oa_sb[64:65, :])
                if rbatches[bi][-1] == jidx:
                    div_stage1(bi, rbatches[bi], dn_tiles[bi])

            # all input DMAs are issued up front (spool/vpool hold one
            # buffer per slot, so no rotation waits). Q|K transfers lead
            # and V transfers trail two slots behind: the cold-phase
            # O-deferral means no V byte is needed until the PE clock-gate
            # has warmed, so the early (DMA-ramp-limited) bandwidth all
            # goes to the S-matmul stream.
            open_qk(0)
            open_qk(1)
            open_qk(2)
            for jidx in range(3, NSLOTS):
                open_v(jidx - 3)
                open_qk(jidx)
            for jidx in range(NSLOTS - 3, NSLOTS):
                open_v(jidx)

            # flat k-tile schedule: exp groups are GK consecutive k-tiles
            # REGARDLESS of slot boundaries, so every ACTIVATE but the
            # last runs at the full N=1536 (23 instead of 26 EXPs - the
            # per-instruction overhead is ~290ns) and slot transitions
            # produce no short-group hiccups
            flat = []
            for jidx, j in enumerate(order):
                w = slots_nk[j]
                for ki in range(w):
                    flat.append((jidx, ki, ki == w - 1))
            # group 0 covers slot 0 alone so the first EXP gates only on
            # the first (smallest) QK transfer
            w0 = min(slots_nk[order[0]], GK)
            fgroups = [flat[:w0]] + [flat[i:i + GK]
                                     for i in range(w0, len(flat), GK)]

            pending = None      # [(pj, ki, closes_unit, ph, p_sb), ...]
            stash = []          # deferred O-groups during the cold phase

            def run_group(items, last=False):
                for pj, ki, closes, ph, p_prev in items:
                    _, pxv, ppo, pw = slot_ctx[pj]
                    pva = pxv.rearrange("p (w c) -> p w c", c=65)
                    nc.tensor.matmul(
                        ppo,
                        lhsT=pva[:, ki, :],
                        rhs=p_prev[:, ph * QB:(ph + 1) * QB],
                        start=(ki == 0), stop=(ki == pw - 1),
                    )
                    if closes:
                        close_slot(pj, last=last)

            # While the PE HAM clock-gate is still cold (1.2 GHz), S
            # matmuls alone (~530ns/k-tile with the row-group pairing)
            # just keep pace with the exp stream, but S+O (~1040ns) would
            # stall it. So O-matmul groups from the first COLD_GROUPS
            # groups are queued and drained two-per-group once the gate
            # has had a full activity window to open - the exp stream
            # never waits on a cold S+O round.
            for gi, grp in enumerate(fgroups):
                ww = len(grp) * QB
                ps = psum_s.tile([128, GK * QB], F32, tag="ps")
                for i, (jidx, ki, _) in enumerate(grp):
                    qk_sb = slot_ctx[jidx][0]
                    qt_sb = qk_sb[:, 0:QB]
                    kt_sb = qk_sb[:, QB:]
                    rg = (ki % 2) * D   # row-group half = k-tile parity
                    nc.tensor.matmul(
                        ps[:, i * QB:(i + 1) * QB],
                        lhsT=kt_sb[rg:rg + D, (ki // 2) * KT:
                                   (ki // 2 + 1) * KT],
                        rhs=qt_sb[rg:rg + D, :],
                        start=True, stop=True,
                        tile_position=(rg, 0),
                    )
                if pending is not None:
                    if gi <= COLD_GROUPS or stash:
                        stash.append(pending)
                        if gi > COLD_GROUPS:
                            for _ in range(2):
                                if stash:
                                    run_group(stash.pop(0))
                    else:
                        run_group(pending)
                p_sb = ppool.tile([128, GK * QB], F16, tag="p")
                nc.scalar.activation(
                    p_sb[:, :ww], ps[:, :ww],
                    AF.Exp, scale=0.125,
                )
                pending = [(jidx, ki, closes, i, p_sb)
                           for i, (jidx, ki, closes) in enumerate(grp)]
            while stash:
                run_group(stash.pop(0))
            run_group(pending, last=True)

    nc.compile()
    return nc


def _pack(queries, keys, values, valid_lens, slots_nk, offs, assign):
    order = _order(slots_nk)
    xw = [sum(_xw(w)) for w in slots_nk]
    tot = sum(xw)
    data = np.zeros((NCORES, 2 * D, tot), NPF16)
    for c in range(NCORES):
        x0 = 0
        for j in order:
            b, qb = assign[c][j]
            w = slots_nk[j]
            wqk, wv = _xw(w)
            vl = int(valid_lens[b])
            blk = data[c, :, x0:x0 + xw[j]]
            qt = queries[b, qb * QB:(qb + 1) * QB, :].T        # [D, QB]
            blk[:D, 0:QB] = qt
            blk[D:, 0:QB] = qt
            # K^T tiles alternate partition halves: even k-tile i in rows
            # 0-63, odd in rows 64-127, both at column block i//2
            for i in range(w):
                half = (i % 2) * D
                c0 = QB + (i // 2) * KT
                blk[half:half + D, c0:c0 + KT] = (
                    keys[b, i * KT:(i + 1) * KT, :].T)
            vv = np.zeros((w * KT, 65), np.float32)
            vv[:vl, :D] = values[b, :vl, :]
            vv[:vl, D] = 1.0
            # [128 partitions, w, 65] flattened on the free axis
            blk[:, wqk:] = (
                vv.reshape(w, KT, 65).transpose(1, 0, 2).reshape(KT, w * 65))
            x0 += xw[j]
    return [{"data": data[c]} for c in range(NCORES)]


def kernel(queries, keys, values, valid_lens):
    global LAST_RESULTS
    queries = np.asarray(queries, dtype=np.float32)
    keys = np.asarray(keys, dtype=np.float32)
    values = np.asarray(values, dtype=np.float32)
    valid_lens = np.asarray(valid_lens)

    key = tuple(int(v) for v in valid_lens)
    if key not in _cache:
        nk, slots_nk, offs, assign = _schedule(valid_lens)
        nc = _build(slots_nk, offs)
        _cache[key] = (nc, slots_nk, offs, assign)
    nc, slots_nk, offs, assign = _cache[key]

    in_maps = _pack(queries, keys, values, valid_lens, slots_nk, offs, assign)
    res = run_bass_kernel_spmd(nc, in_maps, list(range(NCORES)))
    LAST_RESULTS = res

    out = np.empty((B, Q, D), np.float32)
    for c in range(NCORES):
        oc = res.results[c]["out"]          # [NSLOTS, D, QB]
        for j in range(NSLOTS):
            b, qb = assign[c][j]
            out[b, qb * QB:(qb + 1) * QB, :] = oc[j].T
    return out


# revision 59
# speedup vs baseline: 1.0301x; 1.0301x over previous
"""Masked dot-product attention (B=16, Q=K=2048, D=64) on 8 Trainium2 cores.

out = softmax(Q K^T / sqrt(64) + mask(valid_lens)) V, reproducing
reference.py's masked_softmax exactly (to fp16-matmul precision).

Sharding / load balance
-----------------------
Work units are (batch, 512-wide q-block): 64 units whose cost is
nk(b) = ceil(valid_len[b]/128) k-tiles. Units are sorted by nk descending and
dealt round-robin into 8 slots x 8 cores, so every core runs the *same*
static SPMD program (slot j processes NK_j = max-nk-of-its-rank-group
k-tiles) while the host packs each core's own data. Per-core inputs arrive
as packed [128, *] fp16 buffers per slot: [Q^T dup | K^T half-packed] and
[V_aug] (see _xw). Q|K and V ride separate DMAs, with V transfers trailing
so the early (ramp-limited) DMA bandwidth all feeds the S-matmul stream;
the Sync queue carries ONLY input transfers, all epilogue DMAs ride
GpSimd (epilogue producer-waits on Sync were observed head-of-line
blocking V issues and stalling O-matmuls by several us).

Device pipeline (inputs fp16; PSUM accumulates fp32)
----------------------------------------------------
The ACT engine is the roofline: every score flows through one EXP
ACTIVATE at 1 elem/cycle/lane @ 1.2 GHz (~35us/core for 67 k-tiles).
Everything else is structured to keep ACT saturated:
  PE : S^T[128k, 512q] per k-tile = matmul(lhsT=K^T-tile, rhs=Q^T),
       contraction d=64, alternating k-tiles on PE row groups 0-63/64-127
  ACT: P = exp(S^T/8), one ACTIVATE over the 3-bank PSUM group
  PE : O^T_aug[65, 512q] += matmul(lhsT=V_aug-tile[128,65], rhs=P-slice)
O-matmuls lag one group behind S so the PE queue never head-of-line
blocks on exp. V_aug = [V | 1] with rows >= valid_len zeroed by the host
(exact masking, free denominator in row 64). A burst of dependency-free
warm-up matmuls at t=0 plus a cold-phase O-deferral (the first
COLD_GROUPS groups' O-matmuls are stashed and drained two-per-group once
the PE HAM clock-gate has had a full activity window to open to 2.4 GHz;
cold S-matmuls alone keep pace with exp) protect the exp stream from the
1.2 GHz cold clock. The slot order (three smallest first, then largest
descending, biggest last) gets the first EXP issued as early as the
first small DMA allows while the big transfers stream in behind.

Division epilogue
-----------------
Positions 0-5 (processing order): denominator rows are batched on DVE
([3,512] RECIPROCAL x2 - iterative divide is FD-serial, so batching
across partitions amortizes it), broadcast across the 64 d-partitions via
a DRAM-bounce DMA, then multiplied + DMA'd out, all hidden under later
units' exp stream. Positions 6 and 7 instead reciprocate on the ACT
engine as r = exp(-ln(d)) (both functions live in the
natural_log_exp_and_others table set - no table switch; _Bacc pins the
selection), broadcast by a ones-column PE matmul in fp16 (5e-4 rel err,
far under tolerance); their PSUM accumulators stay live to the tail, so
no DVE divide or bounce sits on the critical path after the last exp.
The host transposes O^T -> O while unsharding.
"""

import sys

if "/opt/trn_rl_repo" not in sys.path:
    sys.path.insert(0, "/opt/trn_rl_repo")

import numpy as np

import bass_rust as _bass_rust
import concourse.bass as bass
import concourse.mybir as mybir
import concourse.tile as tile
from concourse import bacc
from concourse.bass_utils import run_bass_kernel_spmd
from concourse.hw_specs import get_activation_tables

B, Q, KLEN, D = 16, 2048, 2048, 64
QB = 512                      # q-block width per work unit
NCORES = 8
NSLOTS = (B * (Q // QB)) // NCORES   # 8 slots per core
KT = 128                      # k-tile height
GK = 3                        # k-tiles per exp/ACT group (3 PSUM banks)
NWARM = 5                     # dependency-free PE warm-up matmuls
COLD_GROUPS = 4               # exp groups whose O-matmuls defer to warm PE
F32 = mybir.dt.float32
F16 = mybir.dt.float16
NPF16 = np.float16
AF = mybir.ActivationFunctionType

LAST_RESULTS = None           # BassKernelResults of the most recent run

_cache: dict = {}


class _Bacc(bacc.Bacc):
    """Bacc whose activation-table fixpoint keeps every ACTIVATE on one
    set: Exp and Ln both live in natural_log_exp_and_others, but the
    default selector binds Exp to exp_and_others and then pays two ~1.3us
    ACT_TABLE_LOAD switches around the tail's Ln. Restricting Exp/Ln to
    the combined set (table order/indices preserved - the set id is the
    index into act_info.json) yields a single load."""

    def insert_act_table_loads(self):
        has_activation = any(
            isinstance(i, mybir.InstActivation)
            for b in self.main_func.blocks
            for i in b.instructions
        )
        if not has_activation:
            return
        tables = []
        for name, fns in get_activation_tables(self.m.arch).items():
            if name != "natural_log_exp_and_others":
                fns = fns - {AF.Exp, AF.Ln}
            tables.append((name, fns))
        _bass_rust.insert_act_table_loads(self, tables)


def _schedule(valid_lens):
    """Static work schedule from valid_lens (host-known at call time)."""
    nk = [max(1, -(-int(v) // KT)) for v in valid_lens]
    units = [(b, qb) for b in range(B) for qb in range(Q // QB)]
    units.sort(key=lambda u: (-nk[u[0]], u))
    slots_nk = [nk[units[NCORES * j][0]] for j in range(NSLOTS)]
    assign = [[units[NCORES * j + c] for j in range(NSLOTS)] for c in range(NCORES)]
    offs = np.concatenate([[0], np.cumsum(slots_nk)]).tolist()
    return nk, slots_nk, offs, assign


def _order(slots_nk):
    """Processing order: two smallest first (their little DMAs land fast,
    so the exp stream starts early), then the rest descending so the
    serial input-DMA stream always runs ahead of compute, and the biggest
    slot last: the two DVE reciprocal batches then have the whole final
    slot's stream to hide under, leaving only the ACT-based division of
    the final unit on the tail."""
    asc = sorted(range(NSLOTS), key=lambda j: (slots_nk[j], j))
    return [asc[0], asc[1], asc[2], asc[6], asc[4], asc[3], asc[5], asc[7]]


def _xw(w):
    """Per-slot packed widths: [Q^T dup | K^T half-packed] + [V_aug].

    K^T tiles alternate between partition halves (even k-tile i in rows
    0-63 at column block i//2, odd in rows 64-127) - half the K DMA bytes
    of a full duplication while keeping the PE row-group pairing; only
    the small Q^T block is duplicated."""
    kk = (w + 1) // 2
    return QB + kk * KT, w * 65


def _build(slots_nk, offs):
    """Build + compile the single SPMD program for the given slot profile."""
    order = _order(slots_nk)
    xw = [sum(_xw(w)) for w in slots_nk]
    xoffs = np.concatenate([[0], np.cumsum([xw[j] for j in order])]).tolist()
    # reciprocal batches over unit positions (processing order); the final
    # two positions divide on ACT instead (see docstring)
    rbatches = [[0, 1, 2], [3, 4, 5]]
    ACT_DIV = (6, 7)

    nc = _Bacc()
    data_d = nc.dram_tensor("data", [2 * D, xoffs[-1]], F16,
                            kind="ExternalInput").ap()
    out_d = nc.dram_tensor("out", [NSLOTS, D, QB], F32, kind="ExternalOutput").ap()

    with tile.TileContext(nc) as tc:
        with (
            tc.tile_pool(name="spool", bufs=8) as spool,
            tc.tile_pool(name="vpool", bufs=8) as vpool,
            tc.tile_pool(name="ppool", bufs=8) as ppool,
            tc.tile_pool(name="epool", bufs=3) as epool,
            tc.tile_pool(name="gpool", bufs=1) as gpool,
            tc.tile_pool(name="opool", bufs=8) as opool,
            tc.tile_pool(name="dpool", bufs=2, space="DRAM") as dpool,
            tc.tile_pool(name="psum_s", bufs=2, space="PSUM") as psum_s,
            tc.tile_pool(name="psum_o", bufs=2, space="PSUM") as psum_o,
        ):
            dn_tiles = {}
            for bi, ub in enumerate(rbatches):
                dn_tiles[bi] = gpool.tile([len(ub), QB], F32, name=f"dn{bi}",
                                          tag=f"dn{bi}")
            ones_sb = gpool.tile([1, D], F16, name="ones", tag="ones")
            nc.vector.memset(ones_sb, 1.0)

            # PE warm-up: dependency-free matmuls from t=0 keep the PE busy
            # through the HAM activity window so the clock-gate opens to
            # 2.4 GHz before (or soon after) the first real matmul.
            warm_row = gpool.tile([1, QB], F16, name="warm_row", tag="warm_row")
            nc.vector.memset(warm_row, 1.0)
            warm_ps = psum_o.tile([65, QB], F32, name="warm_ps", tag="po")
            for _ in range(NWARM):
                nc.tensor.matmul(warm_ps[0:64, :], lhsT=ones_sb, rhs=warm_row,
                                 start=True, stop=True)

            o_tiles = {}
            div_stage2 = []   # (ub, tile) waiting for multiplies

            def div_stage1(bi, ub, dn):
                # reciprocal + partition-broadcast via DRAM bounce (DRAM is
                # flat, so the read-back can replicate across partitions
                # with a stride-0 leading dim), hidden under later units'
                # exp stream. Out-hop on Sync, in-hop on the otherwise-idle
                # GpSimd queue so neither blocks input-DMA issues.
                r_sb = epool.tile([len(ub), QB], F32, tag="r")
                nc.vector.reciprocal(r_sb, dn)
                scratch = dpool.tile([len(ub), QB], F32, tag="scr")
                nc.gpsimd.dma_start(out=scratch, in_=r_sb)
                rb_sb = epool.tile([D, len(ub), QB], F32, tag="rb")
                bcast_src = bass.AP(
                    tensor=scratch.tensor,
                    offset=scratch.offset,
                    ap=[[0, D]] + [list(a) for a in scratch.ap],
                )
                nc.gpsimd.dma_start(out=rb_sb, in_=bcast_src)
                div_stage2.append((list(ub), rb_sb))

            def emit_stage2(ub, rtile):
                for ui, jj in enumerate(ub):
                    oo_sb = opool.tile([D, QB], F32, tag="oo")
                    nc.vector.tensor_mul(oo_sb, o_tiles[jj], rtile[:, ui, :])
                    nc.gpsimd.dma_start(out=out_d[order[jj]], in_=oo_sb)

            def flush_stage2():
                while div_stage2:
                    emit_stage2(*div_stage2.pop(0))

            slot_ctx = {}

            def open_qk(jidx):
                j = order[jidx]
                w = slots_nk[j]
                wqk, wv = _xw(w)
                qk_sb = spool.tile([2 * D, wqk], F16, tag="xqk")
                nc.sync.dma_start(
                    out=qk_sb,
                    in_=data_d[:, xoffs[jidx]:xoffs[jidx] + wqk])
                po = psum_o.tile([65, QB], F32, tag="po")
                slot_ctx[jidx] = [qk_sb, None, po, w]

            def open_v(jidx):
                j = order[jidx]
                w = slots_nk[j]
                wqk, wv = _xw(w)
                xv_sb = vpool.tile([2 * D, wv], F16, tag="xv")
                nc.sync.dma_start(
                    out=xv_sb,
                    in_=data_d[:, xoffs[jidx] + wqk:xoffs[jidx] + wqk + wv])
                slot_ctx[jidx][1] = xv_sb

            act_div_units = []   # (jidx, po) divided on ACT at the tail

            def close_slot(jidx, last=False):
                # flush every pending multiply batch first (their bounce
                # DMAs were issued at least one slot ago), then start this
                # unit's denominator chain as early as possible
                _, _, po, _ = slot_ctx[jidx]
                flush_stage2()
                if jidx in ACT_DIV:
                    # the last two positions divide on the ACT engine after
                    # the exp stream ends: r = exp(-ln(d)) (same activation
                    # table set as exp - no table switch), broadcast across
                    # the 64 d-partitions by a ones-column PE matmul in
                    # fp16. Their PSUM accumulators stay live (nothing
                    # reuses those banks), so only the numerator copy runs
                    # here; no DVE reciprocal or DRAM bounce sits on the
                    # critical tail.
                    oa_sb = gpool.tile([D, QB], F32, name=f"oa{jidx}",
                                       tag=f"oa{jidx}")
                    nc.vector.tensor_copy(oa_sb, po[0:64, :])
                    o_tiles[jidx] = oa_sb
                    act_div_units.append((jidx, po))
                    if last:
                        for jj, pp in act_div_units:
                            lnd = epool.tile([1, QB], F32, tag=f"lnd{jj}")
                            nc.scalar.activation(lnd, pp[64:65, :], AF.Ln)
                            r16 = epool.tile([1, QB], F16, tag=f"r16{jj}")
                            nc.scalar.activation(r16, lnd, AF.Exp,
                                                 scale=-1.0)
                            # broadcast target comes from the psum_o pool:
                            # the rotation lands it exactly on this unit's
                            # own accumulator bank, whose readers (the ln
                            # and the numerator copy) are this chain's own
                            # upstream - taking a psum_s buf here instead
                            # was observed stalling the last unit's
                            # S-matmuls ~2us mid-stream (the scheduler
                            # parks the broadcast consumer at the tail
                            # while the ps pool waits on it)
                            bc = psum_o.tile([65, QB], F32, tag="po")
                            nc.tensor.matmul(bc[0:D, :], lhsT=ones_sb,
                                             rhs=r16, start=True, stop=True)
                            oo_sb = opool.tile([D, QB], F32, tag="oo")
                            nc.vector.tensor_mul(oo_sb, o_tiles[jj],
                                                 bc[0:D, :])
                            nc.gpsimd.dma_start(out=out_d[order[jj]],
                                                in_=oo_sb)
                    return
                bi = next(i for i, ub in enumerate(rbatches) if jidx in ub)
                ri = rbatches[bi].index(jidx)
                # one [65,512] PSUM->SBUF copy moves numerator + denominator
                # together (frees the PSUM bank in one op); the denominator
                # row then hops SBUF->SBUF into the batch tile
                oa_sb = gpool.tile([65, QB], F32, name=f"oa{jidx}",
                                   tag=f"oa{jidx}")
                nc.vector.tensor_copy(oa_sb, po)
                o_tiles[jidx] = oa_sb[0:D, :]
                # the Sync queue carries ONLY input transfers: any epilogue
                # DMA there lets the static scheduler park input issues
                # behind a producer wait (observed stalling V transfers,
                # and with them O-matmuls, by several us)
                nc.gpsimd.dma_start(out=dn_tiles[bi][ri:ri + 1, :],
                                    in_=oa_sb[64:65, :])
                if rbatches[bi][-1] == jidx:
                    div_stage1(bi, rbatches[bi], dn_tiles[bi])

            # all input DMAs are issued up front (spool/vpool hold one
            # buffer per slot, so no rotation waits). Q|K transfers lead
            # and V transfers trail two slots behind: the cold-phase
            # O-deferral means no V byte is needed until the PE clock-gate
            # has warmed, so the early (DMA-ramp-limited) bandwidth all
            # goes to the S-matmul stream.
            open_qk(0)
            open_qk(1)
            open_qk(2)
            for jidx in range(3, NSLOTS):
                open_v(jidx - 3)
                open_qk(jidx)
            for jidx in range(NSLOTS - 3, NSLOTS):
                open_v(jidx)

            # flat k-tile schedule: exp groups are GK consecutive k-tiles
            # REGARDLESS of slot boundaries, so every ACTIVATE but the
            # last runs at the full N=1536 (23 instead of 26 EXPs - the
            # per-instruction overhead is ~290ns) and slot transitions
            # produce no short-group hiccups
            flat = []
            for jidx, j in enumerate(order):
                w = slots_nk[j]
                for ki in range(w):
                    flat.append((jidx, ki, ki == w - 1))
            # group 0 covers slot 0 alone so the first EXP gates only on
            # the first (smallest) QK transfer
            w0 = min(slots_nk[order[0]], GK)
            fgroups = [flat[:w0]] + [flat[i:i + GK]
                                     for i in range(w0, len(flat), GK)]

            pending = None      # [(pj, ki, closes_unit, ph, p_sb), ...]
            stash = []          # deferred O-groups during the cold phase

            def run_group(items, last=False):
                for pj, ki, closes, ph, p_prev in items:
                    _, pxv, ppo, pw = slot_ctx[pj]
                    pva = pxv.rearrange("p (w c) -> p w c", c=65)
                    nc.tensor.matmul(
                        ppo,
                        lhsT=pva[:, ki, :],
                        rhs=p_prev[:, ph * QB:(ph + 1) * QB],
                        start=(ki == 0), stop=(ki == pw - 1),
                    )
                    if closes:
                        close_slot(pj, last=last)

            # While the PE HAM clock-gate is still cold (1.2 GHz), S
            # matmuls alone (~530ns/k-tile with the row-group pairing)
            # just keep pace with the exp stream, but S+O (~1040ns) would
            # stall it. So O-matmul groups from the first COLD_GROUPS
            # groups are queued and drained two-per-group once the gate
            # has had a full activity window to open - the exp stream
            # never waits on a cold S+O round.
            for gi, grp in enumerate(fgroups):
                ww = len(grp) * QB
                ps = psum_s.tile([128, GK * QB], F32, tag="ps")
                for i, (jidx, ki, _) in enumerate(grp):
                    qk_sb = slot_ctx[jidx][0]
                    qt_sb = qk_sb[:, 0:QB]
                    kt_sb = qk_sb[:, QB:]
                    rg = (ki % 2) * D   # row-group half = k-tile parity
                    nc.tensor.matmul(
                        ps[:, i * QB:(i + 1) * QB],
                        lhsT=kt_sb[rg:rg + D, (ki // 2) * KT:
                                   (ki // 2 + 1) * KT],
                        rhs=qt_sb[rg:rg + D, :],
                        start=True, stop=True,
                        tile_position=(rg, 0),
                    )
                if pending is not None:
                    if gi <= COLD_GROUPS or stash:
                        stash.append(pending)
                        if gi > COLD_GROUPS:
                            for _ in range(2):
                                if stash:
                                    run_group(stash.pop(0))
                    else:
                        run_group(pending)
                p_sb = ppool.tile([128, GK * QB], F16, tag="p")
                nc.scalar.activation(
                    p_sb[:, :ww], ps[:, :ww],
                    AF.Exp, scale=0.125,
                )
                pending = [(jidx, ki, closes, i, p_sb)
                           for i, (jidx, ki, closes) in enumerate(grp)]
            while stash:
                run_group(stash.pop(0))
            run_group(pending, last=True)

    nc.compile()
    return nc


def _pack(queries, keys, values, valid_lens, slots_nk, offs, assign):
    order = _order(slots_nk)
    xw = [sum(_xw(w)) for w in slots_nk]
    tot = sum(xw)
    data = np.zeros((NCORES, 2 * D, tot), NPF16)
    for c in range(NCORES):
        x0 = 0
        for j in order:
            b, qb = assign[c][j]
            w = slots_nk[j]
            wqk, wv = _xw(w)
            vl = int(valid_lens[b])
            blk = data[c, :, x0:x0 + xw[j]]
            qt = queries[b, qb * QB:(qb + 1) * QB, :].T        # [D, QB]
            blk[:D, 0:QB] = qt
            blk[D:, 0:QB] = qt
            # K^T tiles alternate partition halves: even k-tile i in rows
            # 0-63, odd in rows 64-127, both at column block i//2
            for i in range(w):
                half = (i % 2) * D
                c0 = QB + (i // 2) * KT
                blk[half:half + D, c0:c0 + KT] = (
                    keys[b, i * KT:(i + 1) * KT, :].T)
            vv = np.zeros((w * KT, 65), np.float32)
            vv[:vl, :D] = values[b, :vl, :]
            vv[:vl, D] = 1.0
            # [128 partitions, w, 65] flattened on the free axis
            blk[:, wqk:] = (
                vv.reshape(w, KT, 65).transpose(1, 0, 2).reshape(KT, w * 65))
            x0 += xw[j]
    return [{"data": data[c]} for c in range(NCORES)]


def kernel(queries, keys, values, valid_lens):
    global LAST_RESULTS
    queries = np.asarray(queries, dtype=np.float32)
    keys = np.asarray(keys, dtype=np.float32)
    values = np.asarray(values, dtype=np.float32)
    valid_lens = np.asarray(valid_lens)

    key = tuple(int(v) for v in valid_lens)
    if key not in _cache:
        nk, slots_nk, offs, assign = _schedule(valid_lens)
        nc = _build(slots_nk, offs)
        _cache[key] = (nc, slots_nk, offs, assign)
    nc, slots_nk, offs, assign = _cache[key]

    in_maps = _pack(queries, keys, values, valid_lens, slots_nk, offs, assign)
    res = run_bass_kernel_spmd(nc, in_maps, list(range(NCORES)))
    LAST_RESULTS = res

    out = np.empty((B, Q, D), np.float32)
    for c in range(NCORES):
        oc = res.results[c]["out"]          # [NSLOTS, D, QB]
        for j in range(NSLOTS):
            b, qb = assign[c][j]
            out[b, qb * QB:(qb + 1) * QB, :] = oc[j].T
    return out
